# revision 12
# baseline (speedup 1.0000x reference)
"""Trainium2 Bass kernel for a 3-branch GCN layer (sum of three GCNConvs).

Math: out[b,t] = sum_k A_k @ (x[b,t] @ W_k) + b_k, with A_k the normalized
adjacency (self loops) of the k-th tiny 25-node graph shared across (B,T).

Instead of the dense [1600x1600] kron operator (one big GEMM, ~395k PE
row-cycles/core), factor into two chained PE stages with NO on-chip
transposes (host pre-transposes x, which is free):

  stage W:  Y[btn, (k,c)] = X[btn, c'] @ [W1|W2|W3]      (K=64, F=192)
  stage A:  out[btn, c]   = sum_k kron(I5, A_k^T) @ Y_k  (K=125, F=64 x3)

Tiles are 125 rows = 5 (b,t) groups x 25 nodes, so the graph contraction
is a fixed 125x125 block-diagonal stationary per branch (~184k PE
row-cycles/core total). PSUM is managed as one 8-bank ring; each bank
holds one tile's Y accumulation region and its out region (psum
accumulation state is bank-granular, so never two accumulation groups
per bank). Y is cast fp32->fp16 by batched pair-copies spread over
DVE/ACT/GPSIMD; out is DMA'd directly from PSUM as fp32.

Data-parallel over batch: 8 batches (2400 bt rows) per core x 8 cores.
Bias is added on the host (typically zero; np.any fast-path).
"""

import sys

import numpy as np

if "/opt/trn_rl_repo" not in sys.path:
    sys.path.insert(0, "/opt/trn_rl_repo")

B, T, NNODES, C = 64, 300, 25, 64
N_CORES = 8
BT_LOC = (B // N_CORES) * T          # 2400 (b,t) rows per core
ROWS_LOC = BT_LOC * NNODES           # 60000 btn rows per core
TILE = 125                           # 5 bt-groups x 25 nodes
NTILES = ROWS_LOC // TILE            # 480
NGRP = NTILES // 4                   # 120 groups of 4 tiles
NCHUNK = 8                           # x input DMA chunks
HALF = NTILES // 2                   # tiles per partition-half (240)
CHW = ROWS_LOC // 2 // NCHUNK        # x chunk width in elements (3750)
BANKC = 512                          # fp32 elems per psum bank partition-row
OOFF = 256                           # out region offset within a bank

_PROGRAM_CACHE = {}
# extra kwargs for run_bass_kernel_spmd (test harness sets trace=True here)
_RUN_KW = {}


def _dense_adj(edge_index_k: np.ndarray) -> np.ndarray:
    """PyG GCNConv normalized dense adjacency A[dst, src] (float64)."""
    row = edge_index_k[0].astype(np.int64)
    col = edge_index_k[1].astype(np.int64)
    loop = np.arange(NNODES, dtype=np.int64)
    row = np.concatenate([row, loop])
    col = np.concatenate([col, loop])
    deg = np.zeros(NNODES, dtype=np.float64)
    np.add.at(deg, col, 1.0)
    dinv = np.where(deg > 0, 1.0 / np.sqrt(deg), 0.0)
    norm = dinv[row] * dinv[col]
    A = np.zeros((NNODES, NNODES), dtype=np.float64)
    np.add.at(A, (col, row), norm)
    return A


def _build_program():
    import concourse.bass as bass
    import concourse.tile as tile
    from concourse import bacc, mybir

    f32 = mybir.dt.float32
    f16 = mybir.dt.float16

    nc = bacc.Bacc(
        "TRN2", target_bir_lowering=False, debug=False, num_devices=N_CORES
    )
    # host-pretransposed x: [128, 30000] fp16; partitions 0-63 = channels of
    # btn rows [0, 30000), partitions 64-127 = channels of rows [30000, 60000)
    xh = nc.dram_tensor("xh", [128, ROWS_LOC // 2], f16, kind="ExternalInput").ap()
    # Wcat duplicated on both partition halves: [128, 192]
    wh = nc.dram_tensor("wh", [128, 3 * C], f16, kind="ExternalInput").ap()
    # three block-diagonal graph stationaries kron(I5, A_k^T): [3, 125, 125]
    ah = nc.dram_tensor("ah", [3, TILE, TILE], f16, kind="ExternalInput").ap()
    # permuted output: dev[p, i, c] = out for btn row 125*i + p, channel c
    dev = nc.dram_tensor("dev", [TILE, NTILES, C], f16, kind="ExternalOutput").ap()

    DEPTH = 1  # software-pipeline distance, in 4-tile groups

    with tile.TileContext(nc) as tc:
        with (
            tc.tile_pool(name="const", bufs=1) as const_pool,
            tc.tile_pool(name="ysb", bufs=3) as ysb_pool,
            tc.tile_pool(name="ostg", bufs=2) as ostg_pool,
            tc.tile_pool(name="ring", bufs=1, space="PSUM") as ring_pool,
        ):
            # the whole of PSUM as one 8-bank ring
            big = ring_pool.tile([128, 8, BANKC], f32, tag="ring", name="ring")

            # constants on the scalar HWDGE queue
            wsb = const_pool.tile([128, 3 * C], f16, tag="wcat")
            nc.scalar.dma_start(wsb[:], wh[:])
            asb = []
            for k in range(3):
                t = const_pool.tile([TILE, TILE], f16, tag=f"a{k}")
                nc.scalar.dma_start(t[:], ah[k])
                asb.append(t)
            # x streamed in NCHUNK big chunks on the sync (SP) queue
            xsb = []
            for ci in range(NCHUNK):
                t = const_pool.tile([128, CHW], f16, tag=f"x{ci}")
                nc.sync.dma_start(t[:], xh[:, ci * CHW : (ci + 1) * CHW])
                xsb.append(t)

            def xchunk(i):
                # lhsT [64, 125] for btn tile i
                if i < HALF:
                    ci, off, p0 = i // 30, (i % 30) * TILE, 0
                else:
                    ii = i - HALF
                    ci, off, p0 = ii // 30, (ii % 30) * TILE, 64
                return xsb[ci][p0 : p0 + 64, off : off + TILE]

            ysbs = {}

            def copy_engine(n):
                # only DVE and ACT can read PSUM
                return nc.scalar if n % 2 == 0 else nc.vector

            ncopies = [0]

            def emit_w(g):
                s0 = (g % 2) * 4
                for t in range(4):
                    i = 4 * g + t
                    p0 = 0 if i < HALF else 64
                    nc.tensor.matmul(
                        big[0:TILE, s0 + t, 0 : 3 * C],
                        xchunk(i), wsb[p0 : p0 + 64, :],
                        start=True, stop=True,
                    )
                ysb = ysb_pool.tile([TILE, 4, 3 * C], f16, tag="y", name="y")
                eng = copy_engine(ncopies[0])
                ncopies[0] += 1
                src = big[0:TILE, s0 : s0 + 4, 0 : 3 * C]
                if eng is nc.scalar:
                    eng.copy(ysb[:], src)
                else:
                    eng.tensor_copy(ysb[:], src)
                ysbs[g] = ysb

            OG = 8  # groups per out staging buffer / DMA
            ostg = {"t": None}

            def emit_a(g):
                s0 = (g % 2) * 4
                ysb = ysbs.pop(g)
                for k in range(3):
                    for t in range(4):
                        nc.tensor.matmul(
                            big[0:TILE, s0 + t, OOFF : OOFF + C],
                            asb[k][:],
                            ysb[0:TILE, t, k * C : (k + 1) * C],
                            start=(k == 0), stop=(k == 2),
                        )
                go = g % OG
                if go == 0:
                    ostg["t"] = ostg_pool.tile(
                        [TILE, 4 * OG, C], f16, tag="ostg", name="ostg"
                    )
                st = ostg["t"]
                eng = copy_engine(ncopies[0])
                ncopies[0] += 1
                src = big[0:TILE, s0 : s0 + 4, OOFF : OOFF + C]
                dst = st[0:TILE, 4 * go : 4 * go + 4, 0:C]
                if eng is nc.scalar:
                    eng.copy(dst, src)
                else:
                    eng.tensor_copy(dst, src)
                if go == OG - 1:
                    blk = g // OG
                    nc.gpsimd.dma_start(
                        dev[:, 4 * OG * blk : 4 * OG * (blk + 1), :], st[:]
                    )

            for g in range(NGRP + DEPTH):
                if g < NGRP:
                    emit_w(g)
                if g >= DEPTH:
                    emit_a(g - DEPTH)

    nc.compile()
    return nc


def kernel(x, edge_index, W1, W2, W3, b1, b2, b3):
    from concourse.bass_utils import run_bass_kernel_spmd

    x = np.asarray(x, dtype=np.float32)
    edge_index = np.asarray(edge_index)
    Ws = [np.asarray(W, dtype=np.float64) for W in (W1, W2, W3)]
    bs = [np.asarray(b, dtype=np.float64) for b in (b1, b2, b3)]

    # host-side operator prep
    Wcat = np.concatenate(Ws, axis=1)  # [64, 192]
    wh = np.vstack([Wcat, Wcat]).astype(np.float16)  # [128, 192]
    ah = np.zeros((3, TILE, TILE), dtype=np.float16)
    for k in range(3):
        Ak = _dense_adj(edge_index[k])
        blk = Ak.T.astype(np.float16)
        for g in range(5):
            ah[k, g * NNODES : (g + 1) * NNODES, g * NNODES : (g + 1) * NNODES] = blk
    bias = np.zeros(C, dtype=np.float64)
    for b in bs:
        bias += b

    if "nc" not in _PROGRAM_CACHE:
        _PROGRAM_CACHE["nc"] = _build_program()
    nc = _PROGRAM_CACHE["nc"]

    # per-core host-transposed x: [64, 60000] -> [128, 30000]
    x16 = x.reshape(N_CORES, ROWS_LOC, C).astype(np.float16)
    in_maps = []
    for i in range(N_CORES):
        xT = np.ascontiguousarray(x16[i].T)  # [64, 60000]
        xhi = np.concatenate(
            [xT[:, : ROWS_LOC // 2], xT[:, ROWS_LOC // 2 :]], axis=0
        )  # [128, 30000]
        in_maps.append({"xh": np.ascontiguousarray(xhi), "wh": wh, "ah": ah})

    res = run_bass_kernel_spmd(nc, in_maps, list(range(N_CORES)), **_RUN_KW)
    _PROGRAM_CACHE["last_result"] = res

    # un-permute: dev[p, i, c] -> out[bt = 5i + g, n2, c], p = 25g + n2
    outs = []
    for i in range(N_CORES):
        d = res.results[i]["dev"].astype(np.float32)
        d = d.reshape(5, NNODES, NTILES, C)
        outs.append(np.transpose(d, (2, 0, 1, 3)).reshape(BT_LOC, NNODES, C))
    out = np.stack(outs, axis=0)
    if np.any(bias):
        out += bias.astype(np.float32)[None, None, None, :]
    return np.ascontiguousarray(
        out.reshape(B, T, NNODES, C)
    )


# revision 13
# speedup vs baseline: 1.5847x; 1.5847x over previous
"""Trainium2 Bass kernel for a 3-branch GCN layer (sum of three GCNConvs).

Math: out[b,t] = sum_k A_k @ (x[b,t] @ W_k) + b_k, with A_k the normalized
adjacency (self loops) of the k-th tiny 25-node graph shared across (B,T).

Instead of the dense [1600x1600] kron operator (one big GEMM, ~395k PE
row-cycles/core), factor into two chained PE stages with NO on-chip
transposes (host pre-transposes x, which is free):

  stage W:  Y[btn, (k,c)] = X[btn, c'] @ [W1|W2|W3]      (K=64, F=192)
  stage A:  out[btn, c]   = sum_k kron(I5, A_k^T) @ Y_k  (K=125, F=64 x3)

Tiles are 125 rows = 5 (b,t) groups x 25 nodes, so the graph contraction
is a fixed 125x125 block-diagonal stationary per branch (~184k PE
row-cycles/core total). PSUM is managed as one 8-bank ring; each bank
holds one tile's Y accumulation region and its out region (psum
accumulation state is bank-granular, so never two accumulation groups
per bank). Y is cast fp32->fp16 by batched pair-copies spread over
DVE/ACT/GPSIMD; out is DMA'd directly from PSUM as fp32.

Data-parallel over batch: 8 batches (2400 bt rows) per core x 8 cores.
Bias is added on the host (typically zero; np.any fast-path).
"""

import sys

import numpy as np

if "/opt/trn_rl_repo" not in sys.path:
    sys.path.insert(0, "/opt/trn_rl_repo")

B, T, NNODES, C = 64, 300, 25, 64
N_CORES = 8
BT_LOC = (B // N_CORES) * T          # 2400 (b,t) rows per core
ROWS_LOC = BT_LOC * NNODES           # 60000 btn rows per core
TILE = 125                           # 5 bt-groups x 25 nodes
NTILES = ROWS_LOC // TILE            # 480
NGRP = NTILES // 4                   # 120 groups of 4 tiles
NCHUNK = 8                           # x input DMA chunks
HALF = NTILES // 2                   # tiles per partition-half (240)
CHW = ROWS_LOC // 2 // NCHUNK        # x chunk width in elements (3750)
BANKC = 512                          # fp32 elems per psum bank partition-row
OOFF = 256                           # out region offset within a bank

_PROGRAM_CACHE = {}
# extra kwargs for run_bass_kernel_spmd (test harness sets trace=True here)
_RUN_KW = {}


def _dense_adj(edge_index_k: np.ndarray) -> np.ndarray:
    """PyG GCNConv normalized dense adjacency A[dst, src] (float64)."""
    row = edge_index_k[0].astype(np.int64)
    col = edge_index_k[1].astype(np.int64)
    loop = np.arange(NNODES, dtype=np.int64)
    row = np.concatenate([row, loop])
    col = np.concatenate([col, loop])
    deg = np.zeros(NNODES, dtype=np.float64)
    np.add.at(deg, col, 1.0)
    dinv = np.where(deg > 0, 1.0 / np.sqrt(deg), 0.0)
    norm = dinv[row] * dinv[col]
    A = np.zeros((NNODES, NNODES), dtype=np.float64)
    np.add.at(A, (col, row), norm)
    return A


def _build_program():
    import concourse.bass as bass
    import concourse.tile as tile
    from concourse import bacc, mybir

    f32 = mybir.dt.float32
    f16 = mybir.dt.float16

    nc = bacc.Bacc(
        "TRN2", target_bir_lowering=False, debug=False, num_devices=N_CORES
    )
    # host-pretransposed x: [128, 30000] fp16; partitions 0-63 = channels of
    # btn rows [0, 30000), partitions 64-127 = channels of rows [30000, 60000)
    xh = nc.dram_tensor("xh", [128, ROWS_LOC // 2], f16, kind="ExternalInput").ap()
    # Wcat duplicated on both partition halves: [128, 192]
    wh = nc.dram_tensor("wh", [128, 3 * C], f16, kind="ExternalInput").ap()
    # three block-diagonal graph stationaries kron(I5, A_k^T): [3, 125, 125]
    ah = nc.dram_tensor("ah", [3, TILE, TILE], f16, kind="ExternalInput").ap()
    # permuted output: dev[p, i, c] = out for btn row 125*i + p, channel c
    dev = nc.dram_tensor("dev", [TILE, NTILES, C], f16, kind="ExternalOutput").ap()

    DEPTH = 1  # software-pipeline distance, in 4-tile groups

    with tile.TileContext(nc) as tc:
        with (
            tc.tile_pool(name="const", bufs=1) as const_pool,
            tc.tile_pool(name="ysb", bufs=3) as ysb_pool,
            tc.tile_pool(name="ostg", bufs=2) as ostg_pool,
            tc.tile_pool(name="ring", bufs=1, space="PSUM") as ring_pool,
        ):
            # the whole of PSUM as one 8-bank ring
            big = ring_pool.tile([128, 8, BANKC], f32, tag="ring", name="ring")

            # constants on the scalar HWDGE queue
            wsb = const_pool.tile([128, 3 * C], f16, tag="wcat")
            nc.scalar.dma_start(wsb[:], wh[:])
            asb = []
            for k in range(3):
                t = const_pool.tile([TILE, TILE], f16, tag=f"a{k}")
                nc.scalar.dma_start(t[:], ah[k])
                asb.append(t)
            # x streamed in NCHUNK big chunks on the sync (SP) queue
            xsb = []
            for ci in range(NCHUNK):
                t = const_pool.tile([128, CHW], f16, tag=f"x{ci}")
                nc.sync.dma_start(t[:], xh[:, ci * CHW : (ci + 1) * CHW])
                xsb.append(t)

            def xchunk(i):
                # lhsT [64, 125] for btn tile i
                if i < HALF:
                    ci, off, p0 = i // 30, (i % 30) * TILE, 0
                else:
                    ii = i - HALF
                    ci, off, p0 = ii // 30, (ii % 30) * TILE, 64
                return xsb[ci][p0 : p0 + 64, off : off + TILE]

            ysbs = {}

            def copy(n, dst, src):
                # only DVE and ACT can read PSUM
                if n % 2 == 0:
                    nc.scalar.copy(dst, src)
                else:
                    nc.vector.tensor_copy(dst, src)

            ncopies = [0]
            NSG = NTILES // 8  # 60 super-groups of 8 tiles

            def emit_w(m):
                # 8 W-matmuls into Y ring (banks 0-5) + 4 pair cast-copies
                ysb = ysb_pool.tile([TILE, 8, 3 * C], f16, tag="y", name="y")
                for h in range(4):
                    for u in range(2):
                        i = 8 * m + 2 * h + u
                        s = i % 6
                        p0 = 0 if i < HALF else 64
                        nc.tensor.matmul(
                            big[0:TILE, s, 0 : 3 * C],
                            xchunk(i), wsb[p0 : p0 + 64, :],
                            start=True, stop=True,
                        )
                    s = (8 * m + 2 * h) % 6
                    copy(
                        ncopies[0],
                        ysb[0:TILE, 2 * h : 2 * h + 2, :],
                        big[0:TILE, s : s + 2, 0 : 3 * C],
                    )
                    ncopies[0] += 1
                ysbs[m] = ysb

            OG = 4  # super-groups per out staging buffer / DMA
            ostg = {"t": None}

            def emit_a(m):
                # 3 accumulating A-matmuls, F=512, out = full bank 6 or 7
                ysb = ysbs.pop(m)
                ob = 6 + (m % 2)
                for k in range(3):
                    nc.tensor.matmul(
                        big[0:TILE, ob, 0:BANKC],
                        asb[k][:],
                        ysb[0:TILE, 0:8, k * C : (k + 1) * C],
                        start=(k == 0), stop=(k == 2),
                    )
                go = m % OG
                if go == 0:
                    ostg["t"] = ostg_pool.tile(
                        [TILE, OG, BANKC], f16, tag="ostg", name="ostg"
                    )
                st = ostg["t"]
                copy(ncopies[0], st[0:TILE, go, 0:BANKC], big[0:TILE, ob, 0:BANKC])
                ncopies[0] += 1
                if go == OG - 1:
                    blk = m // OG
                    nc.gpsimd.dma_start(
                        dev[:, 8 * OG * blk : 8 * OG * (blk + 1), :], st[:]
                    )

            for m in range(NSG + DEPTH):
                if m < NSG:
                    emit_w(m)
                if m >= DEPTH:
                    emit_a(m - DEPTH)

    nc.compile()
    return nc


def kernel(x, edge_index, W1, W2, W3, b1, b2, b3):
    from concourse.bass_utils import run_bass_kernel_spmd

    x = np.asarray(x, dtype=np.float32)
    edge_index = np.asarray(edge_index)
    Ws = [np.asarray(W, dtype=np.float64) for W in (W1, W2, W3)]
    bs = [np.asarray(b, dtype=np.float64) for b in (b1, b2, b3)]

    # host-side operator prep
    Wcat = np.concatenate(Ws, axis=1)  # [64, 192]
    wh = np.vstack([Wcat, Wcat]).astype(np.float16)  # [128, 192]
    ah = np.zeros((3, TILE, TILE), dtype=np.float16)
    for k in range(3):
        Ak = _dense_adj(edge_index[k])
        blk = Ak.T.astype(np.float16)
        for g in range(5):
            ah[k, g * NNODES : (g + 1) * NNODES, g * NNODES : (g + 1) * NNODES] = blk
    bias = np.zeros(C, dtype=np.float64)
    for b in bs:
        bias += b

    if "nc" not in _PROGRAM_CACHE:
        _PROGRAM_CACHE["nc"] = _build_program()
    nc = _PROGRAM_CACHE["nc"]

    # per-core host-transposed x: [64, 60000] -> [128, 30000]
    x16 = x.reshape(N_CORES, ROWS_LOC, C).astype(np.float16)
    in_maps = []
    for i in range(N_CORES):
        xT = np.ascontiguousarray(x16[i].T)  # [64, 60000]
        xhi = np.concatenate(
            [xT[:, : ROWS_LOC // 2], xT[:, ROWS_LOC // 2 :]], axis=0
        )  # [128, 30000]
        in_maps.append({"xh": np.ascontiguousarray(xhi), "wh": wh, "ah": ah})

    res = run_bass_kernel_spmd(nc, in_maps, list(range(N_CORES)), **_RUN_KW)
    _PROGRAM_CACHE["last_result"] = res

    # un-permute: dev[p, i, c] -> out[bt = 5i + g, n2, c], p = 25g + n2
    outs = []
    for i in range(N_CORES):
        d = res.results[i]["dev"].astype(np.float32)
        d = d.reshape(5, NNODES, NTILES, C)
        outs.append(np.transpose(d, (2, 0, 1, 3)).reshape(BT_LOC, NNODES, C))
    out = np.stack(outs, axis=0)
    if np.any(bias):
        out += bias.astype(np.float32)[None, None, None, :]
    return np.ascontiguousarray(
        out.reshape(B, T, NNODES, C)
    )
